# revision 1
# baseline (speedup 1.0000x reference)
"""Trainium2 Bass kernel for the NeuralODE Euler-scan problem.

Math reformulation (per core, local batch BL=512 split into 2 blocks of 256):
  reference: x_{t+1} = x_t + dt*(tanh([x_t, I_t] @ W1 + b1) @ W2 + b2)
  we track the pre-activation y_t = x_t @ W1x + I_t*w1i + b1 resident in PSUM:
      h_t     = tanh(y_t)                               (ACT, psum -> sbuf)
      y_{t+1} = y_t + h_t @ (dt*W2@W1x) + dI_t*w1i + dt*b2@W1x   (PE, accum)
      delta_t = h_t @ (dt*W2)                           (PE -> psum, evac DVE)
  x_t is reconstructed on the host: x_t = x0 + cumsum(delta + dt*b2).

Critical-path trick: y is kept in TWO psum banks of alternating step parity.
tanh_t reads bank[t%2]; the other bank (which tanh_{t-1} finished reading)
receives y_{t+1} = y_{t-1} + inc_{t-1} + inc_t.  Everything except the final
h_t-dependent matmul (the "window" matmul) is applied while tanh_t runs, so
the serial chain per step is just tanh -> one matmul -> tanh.

Layout: transposed + block-diagonal over the 2 batch blocks, so y/h live as
[128 partitions = 2 blocks x 64 hidden, 256 samples].  All recurrence matmuls
run in float32r (full PE speed at N>=256, ~13-bit effective mantissa).

Output: per-step-pair deltas land in [32, 512] psum tiles, are packed by
partition-shifted DVE copies into a [128, 512] stage tile (8 steps) and
DMA'd out raw; the host decodes, adds dt*b2, and cumsums.  The batch dim
(4096) is sharded across the 8 cores; each runs this same program.
"""

import os
import numpy as np

import concourse.bass as bass
from concourse import bacc
import concourse.mybir as mybir
from concourse.tile import TileContext
from concourse import bass_utils

B, T, D, H = 4096, 512, 16, 64
NCORES = 8
BL = B // NCORES          # 512 samples per core
S = BL // 2               # 256 samples per block
NSTEP = T - 1             # 511 Euler steps
GPF = 30                  # dI prefetch group size (510 = 17*30)

f32 = mybir.dt.float32
f32r = mybir.dt.float32r
TANH = mybir.ActivationFunctionType.Tanh


def build_nc(nstep=NSTEP):
    nmmi = nstep - 1                # number of y-update steps (di8 rows)
    nchunk = (nstep + 7) // 8
    nc = bacc.Bacc("TRN2", target_bir_lowering=False, debug=False)

    wzz_d = nc.dram_tensor("wzz", (128, 128), f32r, kind="ExternalInput")
    w1i_d = nc.dram_tensor("w1i", (8, 128), f32r, kind="ExternalInput")
    w2d_d = nc.dram_tensor("w2d", (128, 32), f32r, kind="ExternalInput")
    w1x_d = nc.dram_tensor("w1x", (32, 128), f32, kind="ExternalInput")
    ib_d = nc.dram_tensor("ib", (4, 128), f32, kind="ExternalInput")
    x0t_d = nc.dram_tensor("x0t", (32, S), f32, kind="ExternalInput")
    i0b_d = nc.dram_tensor("i0b", (4, S), f32, kind="ExternalInput")
    di_d = nc.dram_tensor("di", (max(nmmi, 1), 8, S), f32r, kind="ExternalInput")
    out_d = nc.dram_tensor("delta", (nchunk, 128, 512), f32, kind="ExternalOutput")

    with TileContext(nc) as tc:
        with tc.tile_pool(name="consts", bufs=1) as cpool, \
             tc.tile_pool(name="hpool", bufs=4) as hpool, \
             tc.tile_pool(name="dipool", bufs=2) as dipool, \
             tc.tile_pool(name="stpool", bufs=3) as spool, \
             tc.tile_pool(name="ypool", bufs=1, space="PSUM") as ypool, \
             tc.tile_pool(name="dpool", bufs=4, space="PSUM") as dpool:

            def load_const(dram, shape, dtype=f32):
                t_ = cpool.tile(list(shape), dtype, name=dram.name + "_sb")
                nc.sync.dma_start(t_[:, :], dram[:, :])
                return t_

            wzz = load_const(wzz_d, (128, 128), f32r)
            w1i = load_const(w1i_d, (8, 128), f32r)
            w2d = load_const(w2d_d, (128, 32), f32r)
            w1x = load_const(w1x_d, (32, 128))
            ib = load_const(ib_d, (4, 128))
            x0t = load_const(x0t_d, (32, S))
            i0b = load_const(i0b_d, (4, S))

            # both parity banks start at y0 = x0 @ W1x + I0*w1i + b1 (fp32)
            ybank = [ypool.tile([128, S], f32, name="yE"),
                     ypool.tile([128, S], f32, name="yO")]
            for yb in ybank:
                nc.tensor.matmul(yb[:, :], w1x[:, :], x0t[:, :],
                                 start=True, stop=False, skip_group_check=True)
                nc.tensor.matmul(yb[:, :], ib[:, :], i0b[:, :],
                                 start=False, stop=False, skip_group_check=True)

            di_tiles = {}

            def ensure_di(k):
                if k in di_tiles or k * GPF >= nmmi:
                    return
                g0 = k * GPF
                gsz = min(GPF, nmmi - g0)
                til = dipool.tile([8, GPF * S], f32r, tag="di", name=f"di{k}")
                nc.sync.dma_start(
                    til[:, :gsz * S].rearrange("p (g s) -> p g s", s=S),
                    di_d[g0:g0 + gsz, :, :].rearrange("g p s -> p g s"),
                )
                di_tiles[k] = til

            ensure_di(0)
            ensure_di(1)

            h_pair = None
            prev_hs = None
            stage = None
            for t in range(nstep):
                e = t % 2
                u = t // 2
                if e == 0:
                    h_pair = hpool.tile([128, 2 * S], f32r, tag="h", name=f"h{u}")
                if t % 8 == 0:
                    stage = spool.tile([128, 512], f32, tag="stage",
                                       name=f"st{t // 8}")
                    if nstep - t < 8:
                        # partial final chunk: zero-fill so the DMA below
                        # never reads unwritten SBUF
                        nc.any.memset(stage[:, :], 0.0)
                if t % GPF == 0 and t > 0:
                    ensure_di(t // GPF + 1)

                hs = h_pair[:, e * S:(e + 1) * S]
                nc.scalar.activation(hs, ybank[e][:, :], TANH)

                if t < nstep - 1:
                    z = ybank[1 - e]       # receives y_{t+1}
                    # off-window updates: run on PE while tanh_t executes
                    if t >= 1:
                        nc.tensor.matmul(z[:, :], wzz[:, :], prev_hs,
                                         start=False, stop=False,
                                         skip_group_check=True)
                    k, s_ = divmod(t, GPF)
                    dsl = di_tiles[k][:, s_ * S:(s_ + 1) * S]
                    nc.tensor.matmul(z[:, :], w1i[:, :], dsl,
                                     start=False, stop=False,
                                     skip_group_check=True)
                    # window matmul: the only h_t-dependent y update
                    nc.tensor.matmul(z[:, :], wzz[:, :], hs,
                                     start=False,
                                     stop=(t >= nstep - 3),
                                     skip_group_check=True)

                if e == 1 or t == nstep - 1:
                    w = 2 * S if e == 1 else S
                    g = u % 4
                    dps = dpool.tile([32, 512], f32, tag="dps", name=f"dps{u}")
                    nc.tensor.matmul(dps[:, :w], w2d[:, :], h_pair[:, :w],
                                     start=True, stop=True,
                                     skip_group_check=True)
                    # partition-shifted evacuation packs 4 pairs into the
                    # 128-partition stage tile for a full-width DMA
                    nc.vector.tensor_copy(stage[32 * g:32 * g + 32, :w],
                                          dps[:, :w])

                if t % 8 == 7 or t == nstep - 1:
                    c = t // 8
                    nc.sync.dma_start(out_d[c, :, :], stage[:, :])

                prev_hs = hs
    nc.compile()
    return nc


def _host_prep(x0, current_profile, tgrid, W1, b1, W2, b2, nstep=NSTEP):
    """Build the shared constants and per-core inputs."""
    nmmi = nstep - 1
    dt = float(np.mean(np.diff(tgrid.astype(np.float64))))
    W1_64 = W1.astype(np.float64)
    W2_64 = W2.astype(np.float64)
    W1x = W1_64[:D]                      # [16, 64]
    w1iv = W1_64[D]                      # [64]
    M = dt * (W2_64 @ W1x)               # [64, 64]
    b2w = dt * (b2.astype(np.float64) @ W1x)   # [64]

    wzz = np.zeros((128, 128), np.float32)
    wzz[:64, :64] = M
    wzz[64:, 64:] = M
    w1i4 = np.zeros((4, 128), np.float32)
    w1i4[0, :64] = w1iv
    w1i4[1, :64] = b2w
    w1i4[2, 64:] = w1iv
    w1i4[3, 64:] = b2w
    w1i8 = np.concatenate([w1i4, w1i4], axis=0)     # [8, 128]
    w2d = np.zeros((128, 32), np.float32)
    w2d[:64, :16] = dt * W2_64
    w2d[64:, 16:] = dt * W2_64
    w1x_blk = np.zeros((32, 128), np.float32)
    w1x_blk[:16, :64] = W1x
    w1x_blk[16:, 64:] = W1x
    ib = np.zeros((4, 128), np.float32)
    ib[0, :64] = w1iv
    ib[1, :64] = b1
    ib[2, 64:] = w1iv
    ib[3, 64:] = b1
    shared = dict(wzz=wzz, w1i=w1i8, w2d=w2d, w1x=w1x_blk, ib=ib)

    in_maps = []
    for c in range(NCORES):
        xl = np.asarray(x0[c * BL:(c + 1) * BL], np.float32)     # [512, 16]
        Il = np.asarray(current_profile[c * BL:(c + 1) * BL], np.float32)
        x0t = np.zeros((32, S), np.float32)
        x0t[:16] = xl[:S].T
        x0t[16:] = xl[S:].T
        i0b = np.zeros((4, S), np.float32)
        i0b[0] = Il[:S, 0]
        i0b[1] = 1.0
        i0b[2] = Il[S:, 0]
        i0b[3] = 1.0
        dI = Il[:, 1:nmmi + 1] - Il[:, 0:nmmi]                   # [512, nmmi]
        di4 = np.zeros((max(nmmi, 1), 4, S), np.float32)
        if nmmi:
            di4[:, 0, :] = dI[:S].T
            di4[:, 1, :] = 1.0
            di4[:, 2, :] = dI[S:].T
            di4[:, 3, :] = 1.0
        # di8[t] applies both inc_{t-1}'s and inc_t's input terms: rows 0:4
        # are di4[t-1] (zeros at t=0), rows 4:8 are di4[t]
        di8 = np.zeros((max(nmmi, 1), 8, S), np.float32)
        if nmmi:
            di8[1:, 0:4] = di4[:-1]
            di8[:, 4:8] = di4
        in_maps.append(dict(shared, x0t=x0t, i0b=i0b, di=di8))
    return dt, in_maps


def _host_decode(arr, xl, dt, b2, nstep=NSTEP):
    """arr: [nchunk, 128, 512] raw delta chunks for one core -> [BL, nstep+1, D]."""
    nchunk = (nstep + 7) // 8
    d6 = arr.reshape(nchunk, 4, 2, 16, 2, S)       # [c, g, q, d, e, s]
    d6 = d6.transpose(0, 1, 4, 2, 5, 3)            # [c, g, e, q, s, d]
    deltas = d6.reshape(nchunk * 8, BL, D)[:nstep].copy()
    deltas += (np.float32(dt) * b2)[None, None, :].astype(np.float32)
    xs = np.cumsum(deltas, axis=0, dtype=np.float32) + xl[None, :, :]
    out = np.empty((BL, nstep + 1, D), np.float32)
    out[:, 0] = xl
    out[:, 1:] = xs.transpose(1, 0, 2)
    return out


_NC_CACHE = {}


def _get_nc(nstep=NSTEP):
    if nstep not in _NC_CACHE:
        _NC_CACHE[nstep] = build_nc(nstep)
    return _NC_CACHE[nstep]


LAST_RESULTS = None


def kernel(x0, current_profile, t, W1, b1, W2, b2):
    global LAST_RESULTS
    x0 = np.asarray(x0, np.float32)
    current_profile = np.asarray(current_profile, np.float32)
    tgrid = np.asarray(t, np.float32)
    W1 = np.asarray(W1, np.float32)
    b1 = np.asarray(b1, np.float32)
    W2 = np.asarray(W2, np.float32)
    b2 = np.asarray(b2, np.float32)

    dt, in_maps = _host_prep(x0, current_profile, tgrid, W1, b1, W2, b2)
    nc = _get_nc()
    res = bass_utils.run_bass_kernel_spmd(
        nc, in_maps, core_ids=list(range(NCORES)),
        trace=bool(os.environ.get("KERNEL_TRACE")),
    )
    LAST_RESULTS = res

    out = np.empty((B, T, D), np.float32)
    for c in range(NCORES):
        xl = x0[c * BL:(c + 1) * BL]
        out[c * BL:(c + 1) * BL] = _host_decode(
            res.results[c]["delta"], xl, dt, b2)
    return out

